# revision 29
# baseline (speedup 1.0000x reference)
"""Trainium2 Bass kernel for nn_Attn_69776038691596.

reference computes:
    proj     = einsum('bsh,kh->bsk', enc, W) + bias          # (B,S,H)
    energies = einsum('bh,bsh->bs', hid, proj)               # (B,S)
    out      = softmax(energies, axis=0)                     # over batch
Algebraic rewrite (exact in real arithmetic):
    u[b,:] = hid[b,:] @ W          # (B,H)  tiny matmul, done on host
    c[b]   = hid[b,:] . bias       # (B,)
    energies[b,s] = enc[b,s,:] . u[b,:] + c[b]
which turns the 275-GFLOP matmul into a streaming weighted reduction
bound by reading encoder_output from HBM once.

Measured on 8 axon trn2 cores: 106.3-118.0 us HW exec (223.8 us for the
prior fp32/STT kernel; spread is the environment's DMA health, ~290-330
GB/s/core stream rate run-to-run), max abs err 6.9e-3.

vs the first fp32 kernel (223.8 us): enc and u ship as fp16, halving the
HBM stream (512 -> 256 MB, floor ~85-107 us at the per-core HBM share of
300-380 GB/s).  The harness tolerance (2e-2 of max|out| = 1) absorbs
fp16 rounding: measured max abs err 7.0e-3 on the reference inputs.

At fp16 pace no single compute engine can keep up: every accumulating
DVE op (STT / tensor_scalar+accum / custom AFFINE_MUL_REDUCE) runs at
1x (~1.2 us per 128x1024), ScalarE activation+accum is also 1x
(~1.25 us), and a DVE-only or DVE+ScalarE split bottoms out ~115-150 us
(measured 152 us).  So the dot products are split across THREE engines
by s-range:

  PE path (s_loc in [0,128), half the stream): host pre-transposes enc
  to [128 h-part, 8 hc, 128 s] per b.  Per b: 8 accumulating matmuls
  with the enc block as the STATIONARY operand and the u column as the
  MOVING operand (K=128, M=128, N=1) -> E_pe[:, b] in PSUM, base
  partition 0 (HW requires matmul outputs at partition 0/32/64, which
  killed the M=1-per-b variant).  Weight loads (~128 cyc) dominate:
  ~0.63 us/b at 2.4 GHz, ~1.25 us/b even at the 1.2 GHz mid p-state,
  vs the 1.39 us/b DMA pace -- and LDWEIGHTS streams on its own SBUF
  port, overlapping the DVE/ScalarE path.  Output lands softmax-ready
  ([128 s', 64 b], batch on the free axis).

  DVE+ScalarE path (s_loc in [128,256)): partition p = r*64 + b
  (r = s parity), free = (k, h), s_local = 128 + 2k + r.  The static
  u table [128, 1024] fp16 (row p holds u[p%64]) is broadcast-AP'd
  over k.  Per 1-MB chunk (4 s-pairs): one DVE tensor_tensor multiply
  (fp16, 2x_1p mode, ~2.1 us), then 4 reduces over h, rotated
  [ScalarE x3, DVE x1]: ScalarE ~60 us, DVE ~33+19 us total -- under
  the DMA stream.  E accumulates column-wise, one PE transpose at the
  end flips it softmax-ready.  (A 4D [128,4,HC,S] chunk tile variant
  of the PE path silently computed garbage / faulted the device -- APs
  and host packing verified correct offline; keep chunk tiles 3D.
  Tried and reverted: dual-ring streaming (+5 us), mid-stream partial
  epilogue for k<56 (+5-10 us, cross-engine sync bubbles).

  Epilogue: both halves get +c[b] fused with the PSUM->SBUF move on
  the DVE, 64-col softmaxes over the free axis, two output DMAs.

Sharding: split the S axis (2048 -> 8 x 256) across the 8 cores.  The
softmax runs over the batch axis, which every core holds entirely, so
no collectives are needed.
"""
import sys

sys.path.insert(0, "/opt/trn_rl_repo")

import numpy as np

B, S, H = 64, 2048, 1024
N_CORES = 8
S_LOC = S // N_CORES          # 256
S_PE = 128                    # s-columns on the PE path
S_DV = S_LOC - S_PE           # s-columns on the DVE/ScalarE path
NPAIR = S_DV // 2             # 64 s-pairs on the DVE path
KC = 4                        # s-pairs per DVE chunk (1 MB chunks)
NCH = NPAIR // KC             # 16 DVE chunks
HC = H // 128                 # 8 h-chunks of 128 partitions

_CACHE = {}


def build_nc():
    import concourse.bacc as bacc
    import concourse.tile as tile
    from concourse import mybir
    from concourse.masks import make_identity
    from contextlib import ExitStack

    f32 = mybir.dt.float32
    f16 = mybir.dt.float16
    Alu = mybir.AluOpType
    Act = mybir.ActivationFunctionType
    X = mybir.AxisListType.X

    nc = bacc.Bacc("TRN2", target_bir_lowering=False, debug=False,
                   num_devices=N_CORES)
    encD = nc.dram_tensor("encD", [NCH, 128, KC, H], f16,
                          kind="ExternalInput").ap()
    encP = nc.dram_tensor("encP", [B // 4, 128, 4 * HC, S_PE], f16,
                          kind="ExternalInput").ap()
    uD = nc.dram_tensor("uD", [128, 1, H], f16, kind="ExternalInput").ap()
    uP = nc.dram_tensor("uP", [128, HC, B], f16, kind="ExternalInput").ap()
    c2D = nc.dram_tensor("c2D", [128, 128], f32, kind="ExternalInput").ap()
    # outP[s', b] for s_local in [0,128); outD[k, r*64+b] for
    # s_local = 128 + 2k + r
    outP = nc.dram_tensor("outP", [S_PE, B], f32, kind="ExternalOutput").ap()
    outD = nc.dram_tensor("outD", [NPAIR, 128], f32,
                          kind="ExternalOutput").ap()

    with ExitStack() as ctx:
        tc = ctx.enter_context(tile.TileContext(nc))
        singles = ctx.enter_context(tc.tile_pool(name="singles", bufs=1))
        # Buffer depths break the observed ~6.15us/chunk serialization
        # loop (TT(j) WAR-waits on chunk j-3's reduces via the prod
        # ring, and the DMA ring head-of-line waits on the ck buffer
        # that TT frees) -- engines sat at 60% busy while pacing the
        # whole kernel.  ~157 KB/partition total, under the 192 KB cap.
        chunks = ctx.enter_context(tc.tile_pool(name="chunks", bufs=7))
        pechunks = ctx.enter_context(tc.tile_pool(name="pechunks", bufs=7))
        prods = ctx.enter_context(tc.tile_pool(name="prods", bufs=5))
        small = ctx.enter_context(tc.tile_pool(name="small", bufs=1))
        psumE = ctx.enter_context(tc.tile_pool(name="psumE", bufs=1,
                                               space="PSUM"))
        psumT = ctx.enter_context(tc.tile_pool(name="psumT", bufs=1,
                                               space="PSUM"))

        # small dependency-free loads first in ring order
        u1 = singles.tile([128, 1, H], f16, tag="u1")
        nc.sync.dma_start(out=u1, in_=uD)
        u_pe = singles.tile([128, HC, B], f16, tag="u_pe")
        nc.sync.dma_start(out=u_pe, in_=uP)
        c2_sb = singles.tile([128, 128], f32, tag="c2_sb")
        nc.sync.dma_start(out=c2_sb, in_=c2D)

        ident128 = singles.tile([128, 128], f32, tag="ident128")
        make_identity(nc, ident128)
        # Warm the ScalarE activation table: Identity is used all main
        # loop; load Exp now so the epilogue doesn't pay ACT_TABLE_LOAD.
        warm = singles.tile([1, 8], f32, tag="act_warm")
        nc.vector.memset(warm, 0.0)
        nc.scalar.activation(warm, warm, Act.Exp)

        # E[p=(r,b), k] energies for the DVE path (written column-wise
        # by the two reduce engines).
        E = singles.tile([128, NPAIR], f32, tag="E")
        u1b = u1.broadcast_to((128, KC, H))
        # E_pe[s', b] energies for the PE path.
        E_pe = psumE.tile([S_PE, B], f32, tag="E_pe")

        def dve_chunk(j):
            ck = chunks.tile([128, KC, H], f16, tag="ck")
            if j == NCH - 1:
                nc.sync.dma_start(out=ck[:, 0:KC // 2, :],
                                  in_=encD[j][:, 0:KC // 2, :])
                nc.sync.dma_start(out=ck[:, KC // 2:KC, :],
                                  in_=encD[j][:, KC // 2:KC, :])
            else:
                nc.sync.dma_start(out=ck, in_=encD[j])
            prod = prods.tile([128, KC, H], f16, tag="prod")
            if j == NCH - 1:
                for half in range(2):
                    sl = slice(half * (KC // 2), (half + 1) * (KC // 2))
                    nc.vector.tensor_tensor(out=prod[:, sl, :],
                                            in0=ck[:, sl, :],
                                            in1=u1b[:, sl, :], op=Alu.mult)
            else:
                nc.vector.tensor_tensor(out=prod, in0=ck, in1=u1b,
                                        op=Alu.mult)
            for q in range(KC):
                jk = j * KC + q
                psl = prod[:, q, :]
                ecol = E[:, jk:jk + 1]
                # Every DVE op is followed by a pipe DRAIN (~op_dur-266ns)
                # during which the next DVE op cannot issue, so the TT
                # alone effectively costs ~4.3us of DVE time: give the
                # DVE a reduce only on alternate chunks (ScalarE runs
                # 4x1.23=4.9us those chunks; both stay under the ~5.6us
                # DMA group cadence).  Last chunk: alternate engines so
                # the tail's reduces run in parallel.
                if j < NCH - 1:
                    eng = (q == KC - 1 and j % 2 == 1)
                else:
                    eng = (q % 2 == 1)
                if eng:
                    nc.vector.tensor_scalar(out=psl, in0=psl, scalar1=1.0,
                                            scalar2=0.0, op0=Alu.mult,
                                            op1=Alu.add, accum_out=ecol)
                else:
                    nc.scalar.activation(psl, psl, Act.Identity,
                                         accum_out=ecol)

        def pe_chunk(j):
            # 4 b per DMA: 8 KB per-partition descriptors (2 KB ones
            # measurably drag the ring).  Keep everything on the Sync
            # HWDGE ring -- splitting streams across both rings was
            # tried and ran ~5 us slower (rings share the 16 queues).
            ckp = pechunks.tile([128, 4 * HC, S_PE], f16, tag="ckp")
            nc.sync.dma_start(out=ckp, in_=encP[j])
            for i in range(4):
                b = 4 * j + i
                for hc in range(HC):
                    nc.tensor.matmul(E_pe[:, b:b + 1],
                                     lhsT=ckp[:, i * HC + hc, :],
                                     rhs=u_pe[:, hc, b:b + 1],
                                     start=(hc == 0), stop=(hc == HC - 1))

        # the two 1-MB streams ride separate rings with independent
        # consumers, so they flow concurrently without coupling
        # PE chunk first in each group: its consumer (32 matmuls,
        # ~0.9us) never lags, so the ring's head-of-line wait always
        # sits on the slower DVE-path buffer, never both.
        for j in range(NCH):
            pe_chunk(j)
            dve_chunk(j)

        # ---------- epilogue ----------
        def softmax_free(e, lo, hi, tagsuf):
            sl = e[:, lo:hi]
            negm = small.tile([e.shape[0], 1], f32, tag=f"negm{tagsuf}")
            nc.vector.tensor_reduce(negm, sl, axis=X, op=Alu.max,
                                    negate=True)
            ssum = small.tile([e.shape[0], 1], f32, tag=f"ssum{tagsuf}")
            nc.scalar.activation(sl, sl, Act.Exp, bias=negm, scale=1.0,
                                 accum_out=ssum)
            rs = small.tile([e.shape[0], 1], f32, tag=f"rs{tagsuf}")
            nc.vector.reciprocal(rs, ssum)
            nc.vector.tensor_scalar_mul(sl, sl, rs)

        # PE path: E_pe already [s', b] -- fused PSUM->SBUF move + c
        e_pe = singles.tile([S_PE, B], f32, tag="e_pe")
        nc.vector.tensor_add(e_pe, E_pe, c2_sb[:, 0:B])
        softmax_free(e_pe, 0, B, "p")
        nc.sync.dma_start(out=outP, in_=e_pe)

        # DVE path: transpose E, then +c, then per-r softmax
        tp = psumT.tile([NPAIR, 128], f32, tag="tp")
        nc.tensor.transpose(tp, E, ident128)
        e = singles.tile([NPAIR, 128], f32, tag="e")
        nc.vector.tensor_add(e, tp, c2_sb[0:NPAIR, :])
        for r in range(2):
            softmax_free(e, r * 64, (r + 1) * 64, f"d{r}")
        nc.sync.dma_start(out=outD, in_=e)

    nc.compile()
    return nc


def _get_nc():
    if "nc" not in _CACHE:
        _CACHE["nc"] = build_nc()
    return _CACHE["nc"]


def _host_prep(hidden, W, b):
    hid2d = np.asarray(hidden, dtype=np.float32).reshape(B, H)
    Wn = np.asarray(W, dtype=np.float32)
    bn = np.asarray(b, dtype=np.float32).reshape(H)
    u16 = (hid2d @ Wn).astype(np.float16)               # (B, H)
    c = hid2d @ bn                                      # (B,)
    # DVE path: u1[p] = u16[p % 64]  (partition p = r*64 + b)
    uD = np.ascontiguousarray(
        np.concatenate([u16, u16], axis=0)[:, None, :])  # (128, 1, H)
    # PE path: uP[p, hc, b] = u16[b, hc*128 + p]
    uP = np.ascontiguousarray(
        u16.T.reshape(HC, 128, B).transpose(1, 0, 2))    # (128, HC, B)
    c2D = np.ascontiguousarray(
        np.broadcast_to(np.tile(c, 2)[None, :], (128, 128))
        .astype(np.float32))
    return uD, uP, c2D


def run_spmd(hidden, encoder_output, W, b, **spmd_kwargs):
    from concourse.bass_utils import run_bass_kernel_spmd

    nc = _get_nc()
    uD, uP, c2D = _host_prep(hidden, W, b)
    enc16 = np.asarray(encoder_output).astype(np.float16)   # (B, S, H)
    in_maps = []
    for cc in range(N_CORES):
        sl = enc16[:, cc * S_LOC:(cc + 1) * S_LOC, :]
        # PE path: encP[j, p, i, hc, s] = enc[4j+i, base + s, hc*128+p]
        pe = sl[:, 0:S_PE, :]
        encP = np.ascontiguousarray(
            pe.transpose(0, 2, 1).reshape(B, HC, 128, S_PE)
            .transpose(0, 2, 1, 3)
            .reshape(B // 4, 4, 128, HC, S_PE).transpose(0, 2, 1, 3, 4)
            .reshape(B // 4, 128, 4 * HC, S_PE))
        # DVE path: encD[j, r*64+b, q, h] = enc[b, base+S_PE+2*(j*KC+q)+r, h]
        dv = sl[:, S_PE:S_LOC, :]
        a = dv.reshape(B, NPAIR, 2, H).transpose(2, 0, 1, 3)
        a = a.reshape(128, NCH, KC, H).transpose(1, 0, 2, 3)
        in_maps.append({"encD": np.ascontiguousarray(a),
                        "encP": encP, "uD": uD, "uP": uP, "c2D": c2D})
    return run_bass_kernel_spmd(nc, in_maps, core_ids=list(range(N_CORES)),
                                **spmd_kwargs)


def kernel(hidden, encoder_output, W, b):
    res = run_spmd(hidden, encoder_output, W, b)
    parts = []
    for cc in range(N_CORES):
        op = res.results[cc]["outP"]                   # (S_PE, B)
        od = res.results[cc]["outD"].reshape(NPAIR, 2, B)
        left = op.T                                    # (B, S_PE)
        right = od.transpose(2, 0, 1).reshape(B, S_DV)  # s = 2k + r
        parts.append(np.concatenate([left, right], axis=1))
    return np.ascontiguousarray(np.concatenate(parts, axis=1))


# revision 31
# speedup vs baseline: 1.0521x; 1.0521x over previous
"""Trainium2 Bass kernel for nn_Attn_69776038691596.

reference computes:
    proj     = einsum('bsh,kh->bsk', enc, W) + bias          # (B,S,H)
    energies = einsum('bh,bsh->bs', hid, proj)               # (B,S)
    out      = softmax(energies, axis=0)                     # over batch
Algebraic rewrite (exact in real arithmetic):
    u[b,:] = hid[b,:] @ W          # (B,H)  tiny matmul, done on host
    c[b]   = hid[b,:] . bias       # (B,)
    energies[b,s] = enc[b,s,:] . u[b,:] + c[b]
which turns the 275-GFLOP matmul into a streaming weighted reduction
bound by reading encoder_output from HBM once.

Measured on 8 axon trn2 cores: 106.3-118.0 us HW exec (223.8 us for the
prior fp32/STT kernel; spread is the environment's DMA health, ~290-330
GB/s/core stream rate run-to-run), max abs err 6.9e-3.

vs the first fp32 kernel (223.8 us): enc and u ship as fp16, halving the
HBM stream (512 -> 256 MB, floor ~85-107 us at the per-core HBM share of
300-380 GB/s).  The harness tolerance (2e-2 of max|out| = 1) absorbs
fp16 rounding: measured max abs err 7.0e-3 on the reference inputs.

At fp16 pace no single compute engine can keep up: every accumulating
DVE op (STT / tensor_scalar+accum / custom AFFINE_MUL_REDUCE) runs at
1x (~1.2 us per 128x1024), ScalarE activation+accum is also 1x
(~1.25 us), and a DVE-only or DVE+ScalarE split bottoms out ~115-150 us
(measured 152 us).  So the dot products are split across THREE engines
by s-range:

  PE path (s_loc in [0,128), half the stream): host pre-transposes enc
  to [128 h-part, 8 hc, 128 s] per b.  Per b: 8 accumulating matmuls
  with the enc block as the STATIONARY operand and the u column as the
  MOVING operand (K=128, M=128, N=1) -> E_pe[:, b] in PSUM, base
  partition 0 (HW requires matmul outputs at partition 0/32/64, which
  killed the M=1-per-b variant).  Weight loads (~128 cyc) dominate:
  ~0.63 us/b at 2.4 GHz, ~1.25 us/b even at the 1.2 GHz mid p-state,
  vs the 1.39 us/b DMA pace -- and LDWEIGHTS streams on its own SBUF
  port, overlapping the DVE/ScalarE path.  Output lands softmax-ready
  ([128 s', 64 b], batch on the free axis).

  DVE+ScalarE path (s_loc in [128,256)): partition p = r*64 + b
  (r = s parity), free = (k, h), s_local = 128 + 2k + r.  The static
  u table [128, 1024] fp16 (row p holds u[p%64]) is broadcast-AP'd
  over k.  Per 1-MB chunk (4 s-pairs): one DVE tensor_tensor multiply
  (fp16, 2x_1p mode, ~2.1 us), then 4 reduces over h, rotated
  [ScalarE x3, DVE x1]: ScalarE ~60 us, DVE ~33+19 us total -- under
  the DMA stream.  E accumulates column-wise, one PE transpose at the
  end flips it softmax-ready.  (A 4D [128,4,HC,S] chunk tile variant
  of the PE path silently computed garbage / faulted the device -- APs
  and host packing verified correct offline; keep chunk tiles 3D.
  Tried and reverted: dual-ring streaming (+5 us), mid-stream partial
  epilogue for k<56 (+5-10 us, cross-engine sync bubbles).

  Epilogue: both halves get +c[b] fused with the PSUM->SBUF move on
  the DVE, 64-col softmaxes over the free axis, two output DMAs.

Sharding: split the S axis (2048 -> 8 x 256) across the 8 cores.  The
softmax runs over the batch axis, which every core holds entirely, so
no collectives are needed.
"""
import sys

sys.path.insert(0, "/opt/trn_rl_repo")

import numpy as np

B, S, H = 64, 2048, 1024
N_CORES = 8
S_LOC = S // N_CORES          # 256
S_PE = 192                    # s-columns on the PE path
S_DV = S_LOC - S_PE           # s-columns on the DVE/ScalarE path
NPAIR = S_DV // 2             # 64 s-pairs on the DVE path
KC = 4                        # s-pairs per DVE chunk (1 MB chunks)
NCH = NPAIR // KC             # 16 DVE chunks
HC = H // 128                 # 8 h-chunks of 128 partitions

_CACHE = {}


def build_nc():
    import concourse.bacc as bacc
    import concourse.tile as tile
    from concourse import mybir
    from concourse.masks import make_identity
    from contextlib import ExitStack

    f32 = mybir.dt.float32
    f16 = mybir.dt.float16
    Alu = mybir.AluOpType
    Act = mybir.ActivationFunctionType
    X = mybir.AxisListType.X

    nc = bacc.Bacc("TRN2", target_bir_lowering=False, debug=False,
                   num_devices=N_CORES)
    encD = nc.dram_tensor("encD", [NCH, 128, KC, H], f16,
                          kind="ExternalInput").ap()
    encP = nc.dram_tensor("encP", [B // 4, 128, 4 * HC, S_PE], f16,
                          kind="ExternalInput").ap()
    uD = nc.dram_tensor("uD", [128, 1, H], f16, kind="ExternalInput").ap()
    uP = nc.dram_tensor("uP", [128, HC, B], f16, kind="ExternalInput").ap()
    c2D = nc.dram_tensor("c2D", [128, 128], f32, kind="ExternalInput").ap()
    # outP[s', b] for s_local in [0,128); outD[k, r*64+b] for
    # s_local = 128 + 2k + r
    outP = nc.dram_tensor("outP", [S_PE, B], f32, kind="ExternalOutput").ap()
    outD = nc.dram_tensor("outD", [NPAIR, 128], f32,
                          kind="ExternalOutput").ap()

    with ExitStack() as ctx:
        tc = ctx.enter_context(tile.TileContext(nc))
        singles = ctx.enter_context(tc.tile_pool(name="singles", bufs=1))
        # Buffer depths break the observed ~6.15us/chunk serialization
        # loop (TT(j) WAR-waits on chunk j-3's reduces via the prod
        # ring, and the DMA ring head-of-line waits on the ck buffer
        # that TT frees) -- engines sat at 60% busy while pacing the
        # whole kernel.  ~157 KB/partition total, under the 192 KB cap.
        chunks = ctx.enter_context(tc.tile_pool(name="chunks", bufs=6))
        pechunks = ctx.enter_context(tc.tile_pool(name="pechunks", bufs=6))
        prods = ctx.enter_context(tc.tile_pool(name="prods", bufs=4))
        small = ctx.enter_context(tc.tile_pool(name="small", bufs=1))
        psumE = ctx.enter_context(tc.tile_pool(name="psumE", bufs=1,
                                               space="PSUM"))
        psumT = ctx.enter_context(tc.tile_pool(name="psumT", bufs=1,
                                               space="PSUM"))

        # small dependency-free loads first in ring order
        u1 = singles.tile([128, 1, H], f16, tag="u1")
        nc.sync.dma_start(out=u1, in_=uD)
        u_pe = singles.tile([128, HC, B], f16, tag="u_pe")
        nc.sync.dma_start(out=u_pe, in_=uP)
        c2_sb = singles.tile([128, 128], f32, tag="c2_sb")
        nc.sync.dma_start(out=c2_sb, in_=c2D)

        ident128 = singles.tile([128, 128], f32, tag="ident128")
        make_identity(nc, ident128)
        # Warm the ScalarE activation table: Identity is used all main
        # loop; load Exp now so the epilogue doesn't pay ACT_TABLE_LOAD.
        warm = singles.tile([1, 8], f32, tag="act_warm")
        nc.vector.memset(warm, 0.0)
        nc.scalar.activation(warm, warm, Act.Exp)

        # E[p=(r,b), k] energies for the DVE path (written column-wise
        # by the two reduce engines).
        E = singles.tile([128, NPAIR], f32, tag="E")
        u1b = u1.broadcast_to((128, KC, H))
        # E_pe[s', b] energies for the PE path; PSUM has 128
        # partitions, so s' splits into a 128-row and a 64-row tile
        # (both at base partition 0, as the HW requires).
        E_pe = psumE.tile([128, B], f32, tag="E_pe")
        E_pe2 = psumE.tile([S_PE - 128, B], f32, tag="E_pe2")

        def dve_chunk(j):
            ck = chunks.tile([128, KC, H], f16, tag="ck")
            if j == NCH - 1:
                nc.sync.dma_start(out=ck[:, 0:KC // 2, :],
                                  in_=encD[j][:, 0:KC // 2, :])
                nc.sync.dma_start(out=ck[:, KC // 2:KC, :],
                                  in_=encD[j][:, KC // 2:KC, :])
            else:
                nc.sync.dma_start(out=ck, in_=encD[j])
            prod = prods.tile([128, KC, H], f16, tag="prod")
            if j == NCH - 1:
                for half in range(2):
                    sl = slice(half * (KC // 2), (half + 1) * (KC // 2))
                    nc.vector.tensor_tensor(out=prod[:, sl, :],
                                            in0=ck[:, sl, :],
                                            in1=u1b[:, sl, :], op=Alu.mult)
            else:
                nc.vector.tensor_tensor(out=prod, in0=ck, in1=u1b,
                                        op=Alu.mult)
            for q in range(KC):
                jk = j * KC + q
                psl = prod[:, q, :]
                ecol = E[:, jk:jk + 1]
                # last chunk: alternate engines so the tail's reduces
                # run in parallel instead of 3-deep on ScalarE.
                # (Shifting the per-chunk DVE reduce fully to ScalarE
                # was tried and measured slower.)
                eng = ((q == KC - 1) if j < NCH - 1 else (q % 2 == 1))
                if eng:
                    nc.vector.tensor_scalar(out=psl, in0=psl, scalar1=1.0,
                                            scalar2=0.0, op0=Alu.mult,
                                            op1=Alu.add, accum_out=ecol)
                else:
                    nc.scalar.activation(psl, psl, Act.Identity,
                                         accum_out=ecol)

        def pe_chunk(j):
            # 4 b per DMA: 8 KB per-partition descriptors (2 KB ones
            # measurably drag the ring).  Keep everything on the Sync
            # HWDGE ring -- splitting streams across both rings was
            # tried and ran ~5 us slower (rings share the 16 queues).
            ckp = pechunks.tile([128, 4 * HC, S_PE], f16, tag="ckp")
            nc.sync.dma_start(out=ckp, in_=encP[j])
            for i in range(4):
                b = 4 * j + i
                for hc in range(HC):
                    nc.tensor.matmul(E_pe[:, b:b + 1],
                                     lhsT=ckp[:, i * HC + hc, 0:128],
                                     rhs=u_pe[:, hc, b:b + 1],
                                     start=(hc == 0), stop=(hc == HC - 1))
                for hc in range(HC):
                    nc.tensor.matmul(E_pe2[:, b:b + 1],
                                     lhsT=ckp[:, i * HC + hc, 128:S_PE],
                                     rhs=u_pe[:, hc, b:b + 1],
                                     start=(hc == 0), stop=(hc == HC - 1))

        # the two 1-MB streams ride separate rings with independent
        # consumers, so they flow concurrently without coupling
        # PE chunk first in each group: its consumer (32 matmuls,
        # ~0.9us) never lags, so the ring's head-of-line wait always
        # sits on the slower DVE-path buffer, never both.
        for j in range(NCH):
            pe_chunk(2 * j)
            dve_chunk(j)
            pe_chunk(2 * j + 1)

        # ---------- epilogue ----------
        def softmax_free(e, lo, hi, tagsuf):
            sl = e[:, lo:hi]
            negm = small.tile([e.shape[0], 1], f32, tag=f"negm{tagsuf}")
            nc.vector.tensor_reduce(negm, sl, axis=X, op=Alu.max,
                                    negate=True)
            ssum = small.tile([e.shape[0], 1], f32, tag=f"ssum{tagsuf}")
            nc.scalar.activation(sl, sl, Act.Exp, bias=negm, scale=1.0,
                                 accum_out=ssum)
            rs = small.tile([e.shape[0], 1], f32, tag=f"rs{tagsuf}")
            nc.vector.reciprocal(rs, ssum)
            nc.vector.tensor_scalar_mul(sl, sl, rs)

        # PE path: E_pe already [s', b] -- fused PSUM->SBUF move + c
        e_pe = singles.tile([128, B], f32, tag="e_pe")
        nc.vector.tensor_add(e_pe, E_pe, c2_sb[:, 0:B])
        softmax_free(e_pe, 0, B, "p")
        nc.sync.dma_start(out=outP[0:128], in_=e_pe)
        e_pe2 = singles.tile([S_PE - 128, B], f32, tag="e_pe2")
        nc.vector.tensor_add(e_pe2, E_pe2, c2_sb[0:S_PE - 128, 0:B])
        softmax_free(e_pe2, 0, B, "p2")
        nc.sync.dma_start(out=outP[128:S_PE], in_=e_pe2)

        # DVE path: transpose E, then +c, then per-r softmax
        tp = psumT.tile([NPAIR, 128], f32, tag="tp")
        nc.tensor.transpose(tp, E, ident128)
        e = singles.tile([NPAIR, 128], f32, tag="e")
        nc.vector.tensor_add(e, tp, c2_sb[0:NPAIR, :])
        for r in range(2):
            softmax_free(e, r * 64, (r + 1) * 64, f"d{r}")
        nc.sync.dma_start(out=outD, in_=e)

    nc.compile()
    return nc


def _get_nc():
    if "nc" not in _CACHE:
        _CACHE["nc"] = build_nc()
    return _CACHE["nc"]


def _host_prep(hidden, W, b):
    hid2d = np.asarray(hidden, dtype=np.float32).reshape(B, H)
    Wn = np.asarray(W, dtype=np.float32)
    bn = np.asarray(b, dtype=np.float32).reshape(H)
    u16 = (hid2d @ Wn).astype(np.float16)               # (B, H)
    c = hid2d @ bn                                      # (B,)
    # DVE path: u1[p] = u16[p % 64]  (partition p = r*64 + b)
    uD = np.ascontiguousarray(
        np.concatenate([u16, u16], axis=0)[:, None, :])  # (128, 1, H)
    # PE path: uP[p, hc, b] = u16[b, hc*128 + p]
    uP = np.ascontiguousarray(
        u16.T.reshape(HC, 128, B).transpose(1, 0, 2))    # (128, HC, B)
    c2D = np.ascontiguousarray(
        np.broadcast_to(np.tile(c, 2)[None, :], (128, 128))
        .astype(np.float32))
    return uD, uP, c2D


def run_spmd(hidden, encoder_output, W, b, **spmd_kwargs):
    from concourse.bass_utils import run_bass_kernel_spmd

    nc = _get_nc()
    uD, uP, c2D = _host_prep(hidden, W, b)
    enc16 = np.asarray(encoder_output).astype(np.float16)   # (B, S, H)
    in_maps = []
    for cc in range(N_CORES):
        sl = enc16[:, cc * S_LOC:(cc + 1) * S_LOC, :]
        # PE path: encP[j, p, i, hc, s] = enc[4j+i, base + s, hc*128+p]
        pe = sl[:, 0:S_PE, :]
        encP = np.ascontiguousarray(
            pe.transpose(0, 2, 1).reshape(B, HC, 128, S_PE)
            .transpose(0, 2, 1, 3)
            .reshape(B // 4, 4, 128, HC, S_PE).transpose(0, 2, 1, 3, 4)
            .reshape(B // 4, 128, 4 * HC, S_PE))
        # DVE path: encD[j, r*64+b, q, h] = enc[b, base+S_PE+2*(j*KC+q)+r, h]
        dv = sl[:, S_PE:S_LOC, :]
        a = dv.reshape(B, NPAIR, 2, H).transpose(2, 0, 1, 3)
        a = a.reshape(128, NCH, KC, H).transpose(1, 0, 2, 3)
        in_maps.append({"encD": np.ascontiguousarray(a),
                        "encP": encP, "uD": uD, "uP": uP, "c2D": c2D})
    return run_bass_kernel_spmd(nc, in_maps, core_ids=list(range(N_CORES)),
                                **spmd_kwargs)


def kernel(hidden, encoder_output, W, b):
    res = run_spmd(hidden, encoder_output, W, b)
    parts = []
    for cc in range(N_CORES):
        op = res.results[cc]["outP"]                   # (S_PE, B)
        od = res.results[cc]["outD"].reshape(NPAIR, 2, B)
        left = op.T                                    # (B, S_PE)
        right = od.transpose(2, 0, 1).reshape(B, S_DV)  # s = 2k + r
        parts.append(np.concatenate([left, right], axis=1))
    return np.ascontiguousarray(np.concatenate(parts, axis=1))
